# revision 14
# baseline (speedup 1.0000x reference)
"""Chamfer distance kernel for Trainium2, 8 NeuronCores.

Problem: B=4 batches, N=M=8192 points, C=3 coords.
  dist1[b,n] = min_m ||xyz1[b,n]-xyz2[b,m]||^2 ;  dist2[b,m] = min_n ||...||^2

Sharding: 4 batches x 2 directions = 8 perfectly balanced SPMD core tasks.
Each core solves one (query=A[8192], database=B[8192]) brute-force pass.

Per-core algorithm:
  min_m d[n,m] = sq1[n] + min_m( sq2[m] - 2*x[n].y[m] )
The inner expression is computed on the TensorEngine as a K=15 bf16 matmul
(hi/lo bf16 split of coordinates => ~fp32 accuracy at full PE speed;
row order: sq2 splits first, then products) into 512-column PSUM banks.
Row-min runs on the Vector engine as reduce_min over [128,2048] PSUM
megatiles (4 banks, double buffered) - the measured DVE floor on this
part (~1.07 ns/elem/lane).  sq1 stays in wide [128,64] layout and is
added after the min; the output DMA un-permutes the (p,k) query order.
Feature matrices are built wide ([128, 64*128] bf16, one 128x128
transpose chunk per point so lhsT and rhs share partition base 0) and
transposed with PE-transposes + ScalarE copies (both otherwise idle).
"""

import sys
import copy

sys.path.insert(0, "/opt/trn_rl_repo")

import numpy as np

import bass_rust
import concourse.bass as bass
import concourse.tile as tile
from concourse import mybir
from concourse.bass_utils import run_bass_kernel_spmd

F32 = mybir.dt.float32
BF16 = mybir.dt.bfloat16

N = 8192          # query points per core
M = 8192          # database points per core
P = 128           # partitions
Q = N // P        # 64 points per partition stripe (n = p*Q + k)
S = 128           # feature slots per point: one 128x128 transpose chunk per
                  # point, so every feature row lands at partition base 0
                  # (matmul requires lhsT and rhs to share base partition)
K = 15            # matmul contraction rows actually used
CHUNK = 512       # matmul moving free dim (one PSUM bank of fp32)
MEGA = 2048       # reduce tile: 4 chunks, 4 PSUM banks
NROWS = Q         # row-tiles to process (tunable for benchmarking only)


def _split_excess_waits(nc, max_waits=1):
    # This container's walrus codegen only supports a single sem-wait
    # command per instruction ("Too many sync wait commands"). Hoist excess
    # sem waits onto NoOps inserted just before the offender on the same
    # engine (program order preserves blocking semantics).
    n_split = 0
    for f in nc.m.functions:
        for b in f.blocks:
            insts = b.instructions
            for ins in list(insts):
                si = ins.sync_info
                if si is None:
                    continue
                w = list(si.on_wait)
                if len(w) <= max_waits:
                    continue
                idx = insts.index(ins)
                keep = w[-max_waits:]
                extra = w[:-max_waits]
                ins.sync_info = bass_rust.SyncInfo(
                    on_wait=keep, on_update=list(si.on_update)
                )
                for j, wt in enumerate(extra):
                    c = bass_rust.InstNoOp(name=f"{ins.name}-wsplit{j}", ins=[], outs=[])
                    c.engine = ins.engine
                    c.sync_info = bass_rust.SyncInfo(on_wait=[wt], on_update=[])
                    insts.insert(idx + j, c)
                    n_split += 1
    return n_split


def _prep_side(nc, pool, xyz_dram, side):
    """DMA [8192,3] f32 -> wide layout, build bf16 hi/lo feature slots,
    return (feat_wide [128, Q*S] bf16, sq_wide [128, Q] f32 or None).

    Feature slot layout per point (slot index f in [0,32)):
      A side (lhsT rows): 0-2: ones,  3-5: xh,  6-8: xh,  9-11: xl, 12-14: xl
      B side (rhs rows):  0-2: sq h/m/l,  3-5: -2yh, 6-8: -2yl,
                          9-11: -2yh, 12-14: -2yl
    """
    v = nc.vector

    w = pool.tile([P, Q * 3], F32, tag=f"{side}_w")
    nc.sync.dma_start(w[:], xyz_dram.rearrange("(p k) c -> p (k c)", p=P))

    feat = pool.tile([P, Q * S], BF16, tag=f"{side}_feat")
    v.memset(feat[:], 0.0)
    f3 = feat[:].rearrange("p (k s) -> p k s", s=S)

    hi_b = pool.tile([P, Q * 3], BF16, tag=f"{side}_hib")
    v.tensor_copy(hi_b[:], w[:])                       # round to bf16
    hi_f = pool.tile([P, Q * 3], F32, tag=f"{side}_hif")
    v.tensor_copy(hi_f[:], hi_b[:])                    # exact back to f32
    lo_f = pool.tile([P, Q * 3], F32, tag=f"{side}_lof")
    v.tensor_tensor(lo_f[:], w[:], hi_f[:], op=mybir.AluOpType.subtract)
    lo_b = pool.tile([P, Q * 3], BF16, tag=f"{side}_lob")
    v.tensor_copy(lo_b[:], lo_f[:])                    # round residual to bf16
    lo_xf = pool.tile([P, Q * 3], F32, tag=f"{side}_loxf")
    v.tensor_copy(lo_xf[:], lo_b[:])                   # exact f32 of bf16 lo

    # x_hat = hi + lo  (exact in f32; <=18 mantissa bits)
    hat = pool.tile([P, Q * 3], F32, tag=f"{side}_hat")
    v.tensor_tensor(hat[:], hi_f[:], lo_xf[:], op=mybir.AluOpType.add)
    # sq = sum_c x_hat_c^2
    prod = pool.tile([P, Q * 3], F32, tag=f"{side}_prod")
    v.tensor_tensor(prod[:], hat[:], hat[:], op=mybir.AluOpType.mult)
    sq = pool.tile([P, Q], F32, tag=f"{side}_sq")
    v.tensor_reduce(
        sq[:],
        prod[:].rearrange("p (k c) -> p k c", c=3),
        axis=mybir.AxisListType.X,
        op=mybir.AluOpType.add,
    )

    if side == "a":
        v.memset(f3[:, :, 0:3], 1.0)
        v.tensor_copy(f3[:, :, 3:6], hi_b[:].rearrange("p (k c) -> p k c", c=3))
        v.tensor_copy(f3[:, :, 6:9], hi_b[:].rearrange("p (k c) -> p k c", c=3))
        v.tensor_copy(f3[:, :, 9:12], lo_b[:].rearrange("p (k c) -> p k c", c=3))
        v.tensor_copy(f3[:, :, 12:15], lo_b[:].rearrange("p (k c) -> p k c", c=3))
        return feat, sq
    else:
        # -2*hi and -2*lo, exact scalings of bf16 values
        hi3 = hi_f[:].rearrange("p (k c) -> p k c", c=3)
        lo3 = lo_xf[:].rearrange("p (k c) -> p k c", c=3)
        v.tensor_scalar_mul(f3[:, :, 3:6], hi3, -2.0)
        v.tensor_scalar_mul(f3[:, :, 9:12], hi3, -2.0)
        v.tensor_scalar_mul(f3[:, :, 6:9], lo3, -2.0)
        v.tensor_scalar_mul(f3[:, :, 12:15], lo3, -2.0)
        # 3-way bf16 split of sq2 into slots 0..2
        v.tensor_copy(f3[:, :, 0:1], sq[:].rearrange("p (k o) -> p k o", o=1))
        s_hf = pool.tile([P, Q], F32, tag="b_shf")
        v.tensor_copy(s_hf[:], f3[:, :, 0:1])
        r1 = pool.tile([P, Q], F32, tag="b_r1")
        v.tensor_tensor(r1[:], sq[:], s_hf[:], op=mybir.AluOpType.subtract)
        v.tensor_copy(f3[:, :, 1:2], r1[:].rearrange("p (k o) -> p k o", o=1))
        s_mf = pool.tile([P, Q], F32, tag="b_smf")
        v.tensor_copy(s_mf[:], f3[:, :, 1:2])
        r2 = pool.tile([P, Q], F32, tag="b_r2")
        v.tensor_tensor(r2[:], r1[:], s_mf[:], op=mybir.AluOpType.subtract)
        v.tensor_copy(f3[:, :, 2:3], r2[:].rearrange("p (k o) -> p k o", o=1))
        return feat, None


def build_nc(repeat=1):
    import contextlib
    nc = bass.Bass()
    a_xyz = nc.dram_tensor("a_xyz", [N, 3], F32, kind="ExternalInput")
    b_xyz = nc.dram_tensor("b_xyz", [M, 3], F32, kind="ExternalInput")
    ident = nc.dram_tensor("ident", [P, P], BF16, kind="ExternalInput")
    out = nc.dram_tensor("dist", [N], F32, kind="ExternalOutput")

    with tile.TileContext(nc) as tc:
        with contextlib.ExitStack() as stack:
            if repeat > 1:
                stack.enter_context(tc.For_i(0, repeat, 1))
            prep = stack.enter_context(tc.tile_pool(name="prep", bufs=1))
            stage = stack.enter_context(tc.tile_pool(name="stage", bufs=1))
            res = stack.enter_context(tc.tile_pool(name="res", bufs=1))
            feat_a, sq1 = _prep_side(nc, prep, a_xyz, "a")
            feat_b, _ = _prep_side(nc, prep, b_xyz, "b")

            # Transpose wide features into matmul layout: PE transpose
            # (idle TensorE) + ScalarE copies (idle ACT). 128 chunks total.
            id_t = stage.tile([P, P], BF16, tag="ident")
            nc.sync.dma_start(id_t[:], ident[:])
            stA = stage.tile([P, Q * S], BF16, tag="stA")
            stB = stage.tile([P, Q * S], BF16, tag="stB")
            n_chunks = (Q * S) // P                     # 64 per side
            with tc.tile_pool(name="tpsum", bufs=8, space="PSUM") as tpp:
                for t in range(n_chunks):
                    sl = slice(t * P, (t + 1) * P)
                    ta = tpp.tile([P, P], BF16, tag="tps")
                    nc.tensor.transpose(ta[:], feat_a[:, sl], id_t[:])
                    nc.scalar.copy(stA[:, sl], ta[:])
                    tb = tpp.tile([P, P], BF16, tag="tps")
                    nc.tensor.transpose(tb[:], feat_b[:, sl], id_t[:])
                    nc.scalar.copy(stB[:, sl], tb[:])
            pp = stack.enter_context(tc.tile_pool(name="psum", bufs=2, space="PSUM"))

            NMEGA = M // MEGA                           # 4 megatiles per row-tile
            accmini = res.tile([P, Q * NMEGA], F32, tag="accmini")

            # Main loop over 64 row-tiles (query groups)
            for r in range(NROWS):
                lhsT = stA[0:K, r * P : (r + 1) * P]
                for g in range(NMEGA):
                    ps = pp.tile([P, MEGA], F32, tag="ps")
                    for c in range(MEGA // CHUNK):
                        cc = (g * (MEGA // CHUNK) + c) * CHUNK
                        nc.tensor.matmul(
                            ps[:, c * CHUNK : (c + 1) * CHUNK],
                            lhsT, stB[0:K, cc : cc + CHUNK],
                            start=True, stop=True,
                        )
                    nc.vector.tensor_reduce(
                        accmini[:, NMEGA * r + g : NMEGA * r + g + 1],
                        ps[:],
                        axis=mybir.AxisListType.X,
                        op=mybir.AluOpType.min,
                    )

            # emin over the megatile partials, add sq1, write out
            emin = res.tile([P, Q], F32, tag="emin")
            nc.vector.tensor_reduce(
                emin[:],
                accmini[:].rearrange("p (k h) -> p k h", h=NMEGA),
                axis=mybir.AxisListType.X,
                op=mybir.AluOpType.min,
            )
            dist = res.tile([P, Q], F32, tag="dist")
            nc.vector.tensor_tensor(
                dist[:], emin[:], sq1[:], op=mybir.AluOpType.add
            )
            nc.sync.dma_start(out.rearrange("(p k) -> p k", p=P), dist[:])

    _split_excess_waits(nc)
    return nc


_NC_CACHE = {}


def _get_nc(repeat=1):
    if repeat not in _NC_CACHE:
        _NC_CACHE[repeat] = build_nc(repeat)
    return _NC_CACHE[repeat]


def kernel(xyz1, xyz2, _trace=False, _repeat=1):
    xyz1 = np.ascontiguousarray(np.asarray(xyz1, dtype=np.float32))
    xyz2 = np.ascontiguousarray(np.asarray(xyz2, dtype=np.float32))
    B = xyz1.shape[0]
    assert xyz1.shape == (B, N, 3) and xyz2.shape == (B, M, 3)

    nc = _get_nc(_repeat)
    import ml_dtypes
    ident = np.eye(P, dtype=ml_dtypes.bfloat16)
    in_maps = []
    for c in range(2 * B):
        b, d = c % B, c // B
        if d == 0:
            in_maps.append({"a_xyz": xyz1[b], "b_xyz": xyz2[b], "ident": ident})
        else:
            in_maps.append({"a_xyz": xyz2[b], "b_xyz": xyz1[b], "ident": ident})

    res = run_bass_kernel_spmd(
        nc, in_maps, core_ids=list(range(2 * B)), trace=_trace
    )
    dist1 = np.stack([res.results[b]["dist"] for b in range(B)])
    dist2 = np.stack([res.results[B + b]["dist"] for b in range(B)])
    if _trace:
        return (dist1, dist2), res
    return dist1, dist2
